# revision 21
# baseline (speedup 1.0000x reference)
"""Cell-list pairwise distance screen (CellList) for 8 Trainium2 NeuronCores.

Computes the masked dense [N, N] lower-triangular distance matrix:
  out[i, j] = sqrt(|c_i - c_j|^2)  if  j < i, both species valid, d2 <= cutoff^2
            = 0                    otherwise

Strategy (block-sparse + single-matmul d2):
  - Host partitions atoms into 48 spatially-compact 128-atom blocks via a
    balanced 4x4x3 sort-split (equal-count slabs in x, then y, then z).  For
    each row block R it gathers candidate columns {j : dist(j, one of R's
    four 32-atom sub-bboxes) <= cutoff, block(j) <= R} -- a conservative
    superset of all pairs, deduplicated at block level.
  - d2 is produced by ONE tensor-engine matmul per 512-col piece:
      d2[i,j] = ri + rj - 2*ci.cj
    expanded over 2-way bf16 splits of the per-block-translated coordinates
    (local coords ~ +-12 A; dropping the mid*mid product leaves ~2e-4 rms
    d2 error -- measured 2 mask flips vs the f32 reference on the target
    data, ~5e-3 rel err, far under the 2e-2 gate).  15 contraction rows per
    band: 9 split cross products + 3 exact ri splits (x ones) + 3 rj splits.
  - Up to 3 row-block segments pack into one 512-col piece as block-diagonal
    bands (K = 45).  Candidate lists split freely across pieces/cores, so
    the 8 cores get an equal number of nearly-full pieces.
  - PSUM holds d2 directly: per piece one DVE bandpass (select t in (1e-3,
    cutoff^2], else 0) -> one ACT sqrt to fp16 -> DMA out.  Host scatters
    the compacted fp16 values into the full [N, N] f32 zero matrix.
  - Dummy 1-row matmuls at body start keep the PE busy while input DMAs are
    in flight so the tensor engine p-state ramps before the real matmuls.
  - All DRAM tensors are laid out so every per-piece DMA is one contiguous
    block (cheap descriptors): inp [NP*K, P+W], out [NP*P, W].
"""

import threading

import numpy as np

N = 6144
P = 128
NCORES = 8
W = 512  # piece width (one PSUM bank)
KB = 15  # contraction rows per band
BANDS = 3  # bands (row-block segments) per piece
K = KB * BANDS  # 45
NWARM = 3  # PE p-state warmup matmuls
# cross-product split pairs kept from the 2-way bf16 splits (row a, col b);
# the dropped (1,1) mid*mid term is ~2e-4 rms in d2 -- measured 2 mask flips
KEEP = [(0, 0), (0, 1), (1, 0)]

_lock = threading.Lock()
_cache: dict = {}


def _register_ops():
    """Register the fused DVE bandpass op at runtime (visible to table-gen)."""
    import concourse.dve_ops as dve_ops
    from concourse.dve_spec import (
        C0,
        C1,
        Spec,
        Src0,
        Zero,
        _has_src1,
        lower,
        select,
    )
    from concourse.dve_uop import DveOpSpec

    def make(name, body, ref):
        for op in dve_ops.OPS:
            if op.name == name:
                return op
        spec = Spec(body=body, reference=ref)
        row = 1 + len(dve_ops.OPS)
        assert row < 0x20
        shas = {}
        for ver in ("v3", "v4"):
            uops = lower(spec, ver=ver)
            shas[ver] = DveOpSpec(
                name=name, opcode=row, uops=uops, rd1_en=_has_src1(spec)
            ).sha(ver)
        op = dve_ops.DveOp(name, spec, subdim=False, uops_sha=shas)
        dve_ops._SUB_OPCODE_FOR_NAME[name] = row
        dve_ops.OPS.append(op)
        dve_ops.CUSTOM_DVE_SPECS[name] = spec
        return op

    # out = (s0 < in0 < s1) ? in0 : 0
    def band_ref(in0, in1, s0, s1, imm2):
        t = in0.astype(np.float32)
        keep = (t > s0) & (t < s1)
        return np.where(keep, t, 0.0).astype(np.float32)

    bandpass = make(
        "BANDPASS_ANT",
        select((Src0 > C0) & (Src0 < C1), Src0, Zero),
        band_ref,
    )
    return bandpass


WLAST = 416  # width of each core's final (lightest) piece


def _build_program(NP, widths, cuthi):
    import concourse.bacc as bacc
    import concourse.mybir as mybir
    import concourse.tile as tile

    bandpass = _register_ops()

    nc = bacc.Bacc("TRN2", target_bir_lowering=False, debug=False, num_devices=NCORES)
    f32 = mybir.dt.float32
    f16 = mybir.dt.float16
    bf16 = mybir.dt.bfloat16
    PW = P + W  # fused [wts | rhs] block width per piece

    inp = nc.dram_tensor("inp", [K, NP * PW], bf16, kind="ExternalInput")
    out = nc.dram_tensor("out", [NP * P, W], f16, kind="ExternalOutput")

    with tile.TileContext(nc) as tc:
        with (
            tc.tile_pool(name="const", bufs=1) as cpool,
            tc.tile_pool(name="work", bufs=NP) as wpool,
            tc.tile_pool(name="outp", bufs=NP) as spool,
            tc.tile_pool(name="psx", bufs=NP, space="PSUM") as ppx,
            tc.tile_pool(name="psw", bufs=1, space="PSUM") as ppw,
        ):
            cc_t = cpool.tile([P, 2], f32, tag="cc")
            inp_t = cpool.tile([K, NP * PW], bf16, tag="inp")
            warm_t = cpool.tile([P, 2], f16, tag="warm")
            ww_t = cpool.tile([1, PW], bf16, tag="ww")

            # input: pieces are packed side by side per DRAM row; queues
            # pull ROW slices (full-width lines) so each DMA has few, large
            # descriptors.  The DMA rings are descriptor-rate-bound
            # (~30-60ns/line), so 15-17 lines per queue beats 45.  vector's
            # dma is issued before its memsets so it goes out first.
            R1, R2 = 17, 31
            nc.gpsimd.dma_start(inp_t[R2:K, :], inp[R2:K, :])
            nc.sync.dma_start(inp_t[0:R1, :], inp[0:R1, :])
            nc.scalar.dma_start(inp_t[R1:R2, :], inp[R1:R2, :])

            # consts via memset on the otherwise-idle vector queue (no DMA);
            # cuthi is baked per build.  ww is the warmup matmul operand.
            nc.vector.memset(cc_t[:, 0:1], 1e-3)
            nc.vector.memset(cc_t[:, 1:2], float(cuthi))
            nc.vector.memset(ww_t[:], 1.0)

            # warmup matmuls: keep the PE continuously busy while the input
            # DMAs land so the tensor clock leaves its low p-state before the
            # real matmuls.  1 contraction row, full 512-col sweep each.
            wps = ppw.tile([P, 2 * W], f32, tag="wps")
            for i in range(NWARM):
                nc.tensor.matmul(
                    wps[:, (i % 2) * W : (i % 2) * W + W],
                    ww_t[:, 0:P],
                    ww_t[:, P:PW],
                    start=True,
                    stop=True,
                )

            # pull the fp16 ACT sqrt table in (single table: same dtype as
            # the real sqrts below)
            nc.vector.memset(warm_t[:, 0:1], 1.0)
            nc.scalar.sqrt(warm_t[:, 1:2], warm_t[:, 0:1])

            for p in range(NP):
                wp = widths[p]
                t = ppx.tile([P, W], f32, tag="t")
                nc.tensor.matmul(
                    t[:, 0:wp],
                    inp_t[:, p * PW : p * PW + P],
                    inp_t[:, p * PW + P : p * PW + P + wp],
                    start=True,
                    stop=True,
                )
                v = wpool.tile([P, W], f16, tag="v")
                # DVE bandpass: (1e-3 < t < cuthi) ? t : 0  (GPSIMD cannot
                # read PSUM on TRN2, so all bands run on the DVE)
                nc.vector._custom_dve(
                    bandpass,
                    out=v[:, 0:wp],
                    in0=t[:, 0:wp],
                    s0=cc_t[:, 0:1],
                    s1=cc_t[:, 1:2],
                )
                s = spool.tile([P, W], f16, tag="s")
                nc.scalar.sqrt(s[:, 0:wp], v[:, 0:wp])
                # outputs all on sync: its seq is otherwise idle and the
                # gpsimd queue's final DRAIN is ~2us
                nc.sync.dma_start(out[p * P : (p + 1) * P, 0:wp], s[:, 0:wp])

    nc.compile()
    return nc


def _get_program(NP, widths, cuthi):
    with _lock:
        key = f"nc{NP}-{widths}-{float(cuthi)}"
        if key not in _cache:
            _cache[key] = _build_program(NP, widths, cuthi)
    return _cache[key]


def _sort_split(coords):
    """Balanced 4x4x3 equal-count spatial partition; returns permutation."""
    n = len(coords)
    idx = np.argsort(coords[:, 0], kind="stable")
    out = []
    nx, ny, nz = 4, 4, 3
    sx = n // nx
    sy = sx // ny
    sz = sy // nz
    for i in range(nx):
        xi = idx[i * sx : (i + 1) * sx]
        xi = xi[np.argsort(coords[xi, 1], kind="stable")]
        for j in range(ny):
            yj = xi[j * sy : (j + 1) * sy]
            yj = yj[np.argsort(coords[yj, 2], kind="stable")]
            out.append(yj)
    return np.concatenate(out)


def _split2(v32):
    """2-way bf16 split: v32 ~ hi + lo (residual ~ v*2^-18)."""
    import ml_dtypes

    bf = ml_dtypes.bfloat16
    hi = v32.astype(bf)
    lo = (v32 - hi.astype(np.float32)).astype(bf)
    return hi.astype(np.float32), lo.astype(np.float32)


def _split3(v32):
    """Exact 3-way bf16 split: v32 == hi + mid + lo (as f32 sums)."""
    import ml_dtypes

    bf = ml_dtypes.bfloat16
    hi = v32.astype(bf)
    r1 = (v32 - hi.astype(np.float32)).astype(np.float32)
    mid = r1.astype(bf)
    r2 = (r1 - mid.astype(np.float32)).astype(np.float32)
    lo = r2.astype(bf)
    recon = (
        hi.astype(np.float32) + mid.astype(np.float32) + lo.astype(np.float32)
    ).astype(np.float32)
    assert np.array_equal(recon, v32), "bf16 3-way split not exact"
    return hi.astype(np.float32), mid.astype(np.float32), lo.astype(np.float32)


def _prepare(species, coordinates, cutoff):
    """Build per-core in_maps plus host-side scatter indices."""
    import ml_dtypes

    bf = ml_dtypes.bfloat16
    coords = np.asarray(coordinates, dtype=np.float32).reshape(-1, 3).copy()
    n = coords.shape[0]
    assert n == N and n % P == 0, coords.shape
    valid = np.asarray(species).reshape(-1) >= 0
    if not valid.all():
        bad = np.where(~valid)[0]
        far = float(coords[valid].max()) if valid.any() else 0.0
        coords[bad] = (far + 20.0 + 10.0 * np.arange(len(bad), dtype=np.float32))[
            :, None
        ]

    cutf = float(cutoff)
    cut2 = np.float32(cutf) * np.float32(cutf)
    cuthi = np.nextafter(cut2, np.float32(np.inf), dtype=np.float32)
    prune2 = (cutf + 1e-3) ** 2  # conservative host-side pruning radius

    pi = _sort_split(coords)
    cs = coords[pi].astype(np.float32)
    NB = n // P
    blk = np.arange(n) // P

    # candidate columns per row block: within prune radius of any of the
    # block's four 32-atom sub-bboxes (z-sorted slabs), deduped at block level
    cands = []
    for R in range(NB):
        rows = cs[R * P : (R + 1) * P]
        keep = np.zeros(n, bool)
        for s in range(4):
            sub = rows[s * 32 : (s + 1) * 32]
            bmin, bmax = sub.min(0), sub.max(0)
            d = np.maximum(0, np.maximum(bmin[None, :] - cs, cs - bmax[None, :]))
            keep |= (d * d).sum(1) <= prune2
        cands.append(np.where(keep & (blk <= R))[0])

    # greedy pour, largest list first, splitting freely at piece boundaries;
    # each piece holds <= cap cols and <= BANDS row-block segments.  First
    # try a fixed 32-bin schedule whose last 8 bins (one per core) are only
    # WLAST wide, so each core's final pipeline piece is light; fall back to
    # unlimited 512-wide bins if that overflows.
    order = sorted(range(NB), key=lambda R: -len(cands[R]))

    def _pour(caps):
        bins = []  # [space_left, cap, [(R, start, width), ...]]
        cur = None
        for R in order:
            left = len(cands[R])
            s0 = 0
            while left > 0:
                if cur is None or cur[0] == 0 or len(cur[2]) == BANDS:
                    if caps is not None and len(bins) >= len(caps):
                        return None
                    cap = W if caps is None else caps[len(bins)]
                    bins.append([cap, cap, []])
                    cur = bins[-1]
                take = min(left, cur[0])
                cur[2].append((R, s0, take))
                cur[0] -= take
                s0 += take
                left -= take
        return bins

    caps = [W] * (3 * NCORES) + [WLAST] * NCORES
    bins = _pour(caps)
    if bins is not None:
        NP = 4
        widths = (W, W, W, WLAST)
    else:
        bins = _pour(None)
        NP = max(1, -(-len(bins) // NCORES))
        widths = (W,) * NP
    nbins = len(bins)

    # assign bins to cores round-robin by fill; within each core keep the
    # fullest pieces first so the pipeline tail is the lightest piece
    def _deal(bins):
        border = sorted(range(len(bins)), key=lambda i: -(bins[i][1] - bins[i][0]))
        per_core = [[] for _ in range(NCORES)]
        for i, b in enumerate(border):
            per_core[i % NCORES].append(bins[b])
        return per_core

    per_core = _deal(bins)
    if any(
        b[1] - b[0] > widths[p]
        for core in per_core
        for p, b in enumerate(core)
    ):
        # a segment-limited bin landed on a slot narrower than its fill;
        # fall back to uniform 512-wide pieces
        bins = _pour(None)
        NP = max(1, -(-len(bins) // NCORES))
        widths = (W,) * NP
        per_core = _deal(bins)

    PW = P + W
    in_maps = []
    idx_maps = []
    for c in range(NCORES):
        inp_m = np.zeros((K, NP * PW), np.float32)
        idx_m = np.full((NP, P, W), N * N, np.int64)
        for p, (_, _, chlist) in enumerate(per_core[c]):
            wts_m = inp_m[:, p * PW : p * PW + P]
            rhs_m = inp_m[:, p * PW + P : (p + 1) * PW]
            off = 0
            for band, (R, s0, w) in enumerate(chlist):
                rows = cs[R * P : (R + 1) * P]
                bmin, bmax = rows.min(0), rows.max(0)
                tR = ((bmin + bmax) * np.float32(0.5)).astype(np.float32)
                rl = (rows - tR).astype(np.float32)
                cand = cands[R][s0 : s0 + w]
                cl = (cs[cand] - tR).astype(np.float32)
                ri = (
                    (rl[:, 0] * rl[:, 0] + rl[:, 1] * rl[:, 1]) + rl[:, 2] * rl[:, 2]
                ).astype(np.float32)
                rj = (
                    (cl[:, 0] * cl[:, 0] + cl[:, 1] * cl[:, 1]) + cl[:, 2] * cl[:, 2]
                ).astype(np.float32)
                kb = band * KB
                rcol = slice(off, off + w)
                NC = len(KEEP)
                for ci in range(3):
                    rs = _split2(rl[:, ci].copy())
                    csp = _split2(cl[:, ci].copy())
                    for i, (a, bb) in enumerate(KEEP):
                        wa = (np.float32(-2.0) * rs[a]).astype(bf).astype(np.float32)
                        row = kb + ci * NC + i
                        wts_m[row, :] = wa
                        rhs_m[row, rcol] = csp[bb]
                for a, sp in enumerate(_split3(ri.copy())):
                    wts_m[kb + 3 * NC + a, :] = sp
                    rhs_m[kb + 3 * NC + a, rcol] = 1.0
                for bb, sp in enumerate(_split3(rj.copy())):
                    wts_m[kb + 3 * NC + 3 + bb, :] = 1.0
                    rhs_m[kb + 3 * NC + 3 + bb, rcol] = sp
                # scatter indices: orig (hi, lo) pair -> tril slot; self -> scratch
                ro = pi[R * P : (R + 1) * P]
                co = pi[cand]
                hi = np.maximum(ro[:, None], co[None, :])
                lo = np.minimum(ro[:, None], co[None, :])
                flat = hi * N + lo
                flat[ro[:, None] == co[None, :]] = N * N
                idx_m[p, :, off : off + w] = flat
                off += w
        in_maps.append({"inp": np.ascontiguousarray(inp_m).astype(bf)})
        idx_maps.append(idx_m)
    with _lock:
        _cache["cuthi"] = float(cuthi)
        _cache["widths"] = tuple(widths)
    return in_maps, idx_maps, NP


def _prepare_inputs(species, coordinates, cutoff):
    in_maps, idx_maps, NP = _prepare(species, coordinates, cutoff)
    return in_maps


def _run(in_maps, trace=False):
    from concourse import bass_utils

    NP = in_maps[0]["inp"].shape[1] // (P + W)
    with _lock:
        cuthi = _cache["cuthi"]
        widths = _cache["widths"]
    nc = _get_program(NP, widths, cuthi)
    return bass_utils.run_bass_kernel_spmd(
        nc, in_maps, core_ids=list(range(NCORES)), trace=trace
    )


def _assemble(results, idx_maps):
    full = np.zeros(N * N + 1, np.float32)
    for c in range(NCORES):
        vals = results[c]["out"].astype(np.float32)
        full[idx_maps[c].ravel()] = vals.ravel()
    return full[: N * N].reshape(N, N)


def kernel(species, coordinates, cutoff):
    in_maps, idx_maps, NP = _prepare(species, coordinates, cutoff)
    res = _run(in_maps)
    return _assemble(res.results, idx_maps)


# revision 27
# speedup vs baseline: 1.2222x; 1.2222x over previous
"""Cell-list pairwise distance screen (CellList) for 8 Trainium2 NeuronCores.

Computes the masked dense [N, N] lower-triangular distance matrix:
  out[i, j] = sqrt(|c_i - c_j|^2)  if  j < i, both species valid, d2 <= cutoff^2
            = 0                    otherwise

Strategy (block-sparse + single-matmul d2):
  - Host partitions atoms into 48 spatially-compact 128-atom blocks via a
    balanced 4x4x3 sort-split (equal-count slabs in x, then y, then z).  For
    each row block R it gathers candidate columns {j : dist(j, one of R's
    four 32-atom sub-bboxes) <= cutoff, block(j) <= R} -- a conservative
    superset of all pairs, deduplicated at block level.
  - d2 is produced by ONE tensor-engine matmul per 512-col piece:
      d2[i,j] = ri + rj - 2*ci.cj
    expanded over 2-way bf16 splits of the per-block-translated coordinates
    (local coords ~ +-12 A; dropping the mid*mid product leaves ~2e-4 rms
    d2 error -- measured 2 mask flips vs the f32 reference on the target
    data, ~5e-3 rel err, far under the 2e-2 gate).  15 contraction rows per
    band: 9 split cross products + 3 exact ri splits (x ones) + 3 rj splits.
  - Up to 3 row-block segments pack into one 512-col piece as block-diagonal
    bands (K = 45).  Candidate lists split freely across pieces/cores, so
    the 8 cores get an equal number of nearly-full pieces.
  - PSUM holds d2 directly: per piece one DVE bandpass (select t in (1e-3,
    cutoff^2], else 0) -> one ACT sqrt to fp16 -> DMA out.  Host scatters
    the compacted fp16 values into the full [N, N] f32 zero matrix.
  - Dummy 1-row matmuls at body start keep the PE busy while input DMAs are
    in flight so the tensor engine p-state ramps before the real matmuls.
  - All DRAM tensors are laid out so every per-piece DMA is one contiguous
    block (cheap descriptors): inp [NP*K, P+W], out [NP*P, W].
"""

import threading

import numpy as np

N = 6144
P = 128
NCORES = 8
W = 512  # piece width (one PSUM bank)
KB = 15  # contraction rows per band
BANDS = 3  # bands (row-block segments) per piece
K = KB * BANDS  # 45
NWARM = 3  # PE p-state warmup matmuls
# cross-product split pairs kept from the 2-way bf16 splits (row a, col b);
# the dropped (1,1) mid*mid term is ~2e-4 rms in d2 -- measured 2 mask flips
KEEP = [(0, 0), (0, 1), (1, 0)]

_lock = threading.Lock()
_cache: dict = {}


def _register_ops():
    """Register the fused DVE bandpass op at runtime (visible to table-gen)."""
    import concourse.dve_ops as dve_ops
    from concourse.dve_spec import (
        C0,
        C1,
        Spec,
        Src0,
        Zero,
        _has_src1,
        lower,
        select,
    )
    from concourse.dve_uop import DveOpSpec

    def make(name, body, ref):
        for op in dve_ops.OPS:
            if op.name == name:
                return op
        spec = Spec(body=body, reference=ref)
        row = 1 + len(dve_ops.OPS)
        assert row < 0x20
        shas = {}
        for ver in ("v3", "v4"):
            uops = lower(spec, ver=ver)
            shas[ver] = DveOpSpec(
                name=name, opcode=row, uops=uops, rd1_en=_has_src1(spec)
            ).sha(ver)
        op = dve_ops.DveOp(name, spec, subdim=False, uops_sha=shas)
        dve_ops._SUB_OPCODE_FOR_NAME[name] = row
        dve_ops.OPS.append(op)
        dve_ops.CUSTOM_DVE_SPECS[name] = spec
        return op

    # out = (s0 < in0 < s1) ? in0 : 0
    def band_ref(in0, in1, s0, s1, imm2):
        t = in0.astype(np.float32)
        keep = (t > s0) & (t < s1)
        return np.where(keep, t, 0.0).astype(np.float32)

    bandpass = make(
        "BANDPASS_ANT",
        select((Src0 > C0) & (Src0 < C1), Src0, Zero),
        band_ref,
    )
    return bandpass


WLAST = 416  # width of each core's final (lightest) piece


def _build_program(NP, widths, cuthi):
    import concourse.bacc as bacc
    import concourse.mybir as mybir
    import concourse.tile as tile

    nc = bacc.Bacc("TRN2", target_bir_lowering=False, debug=False, num_devices=NCORES)
    f32 = mybir.dt.float32
    f16 = mybir.dt.float16
    bf16 = mybir.dt.bfloat16
    PW = P + W  # fused [wts | rhs] block width per piece

    inp = nc.dram_tensor("inp", [K, NP * PW], bf16, kind="ExternalInput")
    out = nc.dram_tensor("out", [NP * P, W], f16, kind="ExternalOutput")

    with tile.TileContext(nc) as tc:
        with (
            tc.tile_pool(name="const", bufs=1) as cpool,
            tc.tile_pool(name="work", bufs=NP) as wpool,
            tc.tile_pool(name="outp", bufs=NP) as spool,
            tc.tile_pool(name="psx", bufs=NP, space="PSUM") as ppx,
            tc.tile_pool(name="psw", bufs=1, space="PSUM") as ppw,
        ):
            inp_t = cpool.tile([K, NP * PW], bf16, tag="inp")
            warm_t = cpool.tile([P, 2], f32, tag="warm")
            ww_t = cpool.tile([1, PW], bf16, tag="ww")

            # input: pieces are packed side by side per DRAM row, so each
            # queue pulls its column half in ONE DMA with 2560B lines.
            # (Lines wider than ~2560B hit a per-partition write-bandwidth
            # cliff: 5120B lines measured ~8x slower per line.)
            HP = (NP + 1) // 2 * PW
            nc.sync.dma_start(inp_t[:, 0:HP], inp[:, 0:HP])
            nc.scalar.dma_start(inp_t[:, HP : NP * PW], inp[:, HP : NP * PW])

            # ww is the warmup matmul operand (idle vector queue, no DMA)
            nc.vector.memset(ww_t[:], 1.0)

            # warmup matmuls: keep the PE continuously busy while the input
            # DMAs land so the tensor clock leaves its low p-state before the
            # real matmuls.  1 contraction row, full 512-col sweep each.
            wps = ppw.tile([P, 2 * W], f32, tag="wps")
            for i in range(NWARM):
                nc.tensor.matmul(
                    wps[:, (i % 2) * W : (i % 2) * W + W],
                    ww_t[:, 0:P],
                    ww_t[:, P:PW],
                    start=True,
                    stop=True,
                )

            # pull the ACT sqrt table in (f32, same dtype as the real
            # sqrts below)
            nc.vector.memset(warm_t[:, 0:1], 1.0)
            nc.scalar.sqrt(warm_t[:, 1:2], warm_t[:, 0:1])

            # sqrt FIRST (ACT reads PSUM directly, so it starts right after
            # each matmul), then the band on the DVE in the sqrt domain:
            # keep dist iff dist < T5 where T5 = nextafter(5.0).  sqrt is
            # monotone, so this is equivalent to d2 <= cutoff^2 up to a
            # ~5e-6-wide d2 window (expected flips ~0.04 pairs).  Self
            # pairs (d2 ~ -1e-5) give sqrt -> NaN -> NaN*0 or NaN kept;
            # either way they scatter to the scratch slot.
            T5 = float(np.nextafter(np.float32(cuthi) ** np.float32(0.5), np.float32(np.inf)))
            for p in range(NP):
                wp = widths[p]
                t = ppx.tile([P, W], f32, tag="t")
                nc.tensor.matmul(
                    t[:, 0:wp],
                    inp_t[:, p * PW : p * PW + P],
                    inp_t[:, p * PW + P : p * PW + P + wp],
                    start=True,
                    stop=True,
                )
                d = wpool.tile([P, W], f32, tag="d")
                nc.scalar.sqrt(d[:, 0:wp], t[:, 0:wp])
                s = spool.tile([P, W], f16, tag="s")
                nc.vector.scalar_tensor_tensor(
                    s[:, 0:wp],
                    d[:, 0:wp],
                    T5,
                    d[:, 0:wp],
                    mybir.AluOpType.is_lt,
                    mybir.AluOpType.mult,
                )
                # outputs all on sync: its seq is otherwise idle and the
                # gpsimd queue's final DRAIN is ~2us
                nc.sync.dma_start(out[p * P : (p + 1) * P, 0:wp], s[:, 0:wp])

    nc.compile()
    return nc


def _get_program(NP, widths, cuthi):
    with _lock:
        key = f"nc{NP}-{widths}-{float(cuthi)}"
        if key not in _cache:
            _cache[key] = _build_program(NP, widths, cuthi)
    return _cache[key]


def _sort_split(coords):
    """Balanced 4x4x3 equal-count spatial partition; returns permutation."""
    n = len(coords)
    idx = np.argsort(coords[:, 0], kind="stable")
    out = []
    nx, ny, nz = 4, 4, 3
    sx = n // nx
    sy = sx // ny
    sz = sy // nz
    for i in range(nx):
        xi = idx[i * sx : (i + 1) * sx]
        xi = xi[np.argsort(coords[xi, 1], kind="stable")]
        for j in range(ny):
            yj = xi[j * sy : (j + 1) * sy]
            yj = yj[np.argsort(coords[yj, 2], kind="stable")]
            out.append(yj)
    return np.concatenate(out)


def _split2(v32):
    """2-way bf16 split: v32 ~ hi + lo (residual ~ v*2^-18)."""
    import ml_dtypes

    bf = ml_dtypes.bfloat16
    hi = v32.astype(bf)
    lo = (v32 - hi.astype(np.float32)).astype(bf)
    return hi.astype(np.float32), lo.astype(np.float32)


def _split3(v32):
    """Exact 3-way bf16 split: v32 == hi + mid + lo (as f32 sums)."""
    import ml_dtypes

    bf = ml_dtypes.bfloat16
    hi = v32.astype(bf)
    r1 = (v32 - hi.astype(np.float32)).astype(np.float32)
    mid = r1.astype(bf)
    r2 = (r1 - mid.astype(np.float32)).astype(np.float32)
    lo = r2.astype(bf)
    recon = (
        hi.astype(np.float32) + mid.astype(np.float32) + lo.astype(np.float32)
    ).astype(np.float32)
    assert np.array_equal(recon, v32), "bf16 3-way split not exact"
    return hi.astype(np.float32), mid.astype(np.float32), lo.astype(np.float32)


def _prepare(species, coordinates, cutoff):
    """Build per-core in_maps plus host-side scatter indices."""
    import ml_dtypes

    bf = ml_dtypes.bfloat16
    coords = np.asarray(coordinates, dtype=np.float32).reshape(-1, 3).copy()
    n = coords.shape[0]
    assert n == N and n % P == 0, coords.shape
    valid = np.asarray(species).reshape(-1) >= 0
    if not valid.all():
        bad = np.where(~valid)[0]
        far = float(coords[valid].max()) if valid.any() else 0.0
        coords[bad] = (far + 20.0 + 10.0 * np.arange(len(bad), dtype=np.float32))[
            :, None
        ]

    cutf = float(cutoff)
    cut2 = np.float32(cutf) * np.float32(cutf)
    cuthi = np.nextafter(cut2, np.float32(np.inf), dtype=np.float32)
    prune2 = (cutf + 1e-3) ** 2  # conservative host-side pruning radius

    pi = _sort_split(coords)
    cs = coords[pi].astype(np.float32)
    NB = n // P
    blk = np.arange(n) // P

    # candidate columns per row block: within prune radius of any of the
    # block's four 32-atom sub-bboxes (z-sorted slabs), deduped at block level
    cands = []
    for R in range(NB):
        rows = cs[R * P : (R + 1) * P]
        keep = np.zeros(n, bool)
        for s in range(4):
            sub = rows[s * 32 : (s + 1) * 32]
            bmin, bmax = sub.min(0), sub.max(0)
            d = np.maximum(0, np.maximum(bmin[None, :] - cs, cs - bmax[None, :]))
            keep |= (d * d).sum(1) <= prune2
        cands.append(np.where(keep & (blk <= R))[0])

    # greedy pour, largest list first, splitting freely at piece boundaries;
    # each piece holds <= cap cols and <= BANDS row-block segments.  First
    # try a fixed 32-bin schedule whose last 8 bins (one per core) are only
    # WLAST wide, so each core's final pipeline piece is light; fall back to
    # unlimited 512-wide bins if that overflows.
    order = sorted(range(NB), key=lambda R: -len(cands[R]))

    def _pour(caps):
        bins = []  # [space_left, cap, [(R, start, width), ...]]
        cur = None
        for R in order:
            left = len(cands[R])
            s0 = 0
            while left > 0:
                if cur is None or cur[0] == 0 or len(cur[2]) == BANDS:
                    if caps is not None and len(bins) >= len(caps):
                        return None
                    cap = W if caps is None else caps[len(bins)]
                    bins.append([cap, cap, []])
                    cur = bins[-1]
                take = min(left, cur[0])
                cur[2].append((R, s0, take))
                cur[0] -= take
                s0 += take
                left -= take
        return bins

    caps = [W] * (3 * NCORES) + [WLAST] * NCORES
    bins = _pour(caps)
    if bins is not None:
        NP = 4
        widths = (W, W, W, WLAST)
    else:
        bins = _pour(None)
        NP = max(1, -(-len(bins) // NCORES))
        widths = (W,) * NP
    nbins = len(bins)

    # assign bins to cores round-robin by fill; within each core keep the
    # fullest pieces first so the pipeline tail is the lightest piece
    def _deal(bins):
        border = sorted(range(len(bins)), key=lambda i: -(bins[i][1] - bins[i][0]))
        per_core = [[] for _ in range(NCORES)]
        for i, b in enumerate(border):
            per_core[i % NCORES].append(bins[b])
        return per_core

    per_core = _deal(bins)
    if any(
        b[1] - b[0] > widths[p]
        for core in per_core
        for p, b in enumerate(core)
    ):
        # a segment-limited bin landed on a slot narrower than its fill;
        # fall back to uniform 512-wide pieces
        bins = _pour(None)
        NP = max(1, -(-len(bins) // NCORES))
        widths = (W,) * NP
        per_core = _deal(bins)

    PW = P + W
    in_maps = []
    idx_maps = []
    for c in range(NCORES):
        inp_m = np.zeros((K, NP * PW), np.float32)
        idx_m = np.full((NP, P, W), N * N, np.int64)
        for p, (_, _, chlist) in enumerate(per_core[c]):
            wts_m = inp_m[:, p * PW : p * PW + P]
            rhs_m = inp_m[:, p * PW + P : (p + 1) * PW]
            off = 0
            for band, (R, s0, w) in enumerate(chlist):
                rows = cs[R * P : (R + 1) * P]
                bmin, bmax = rows.min(0), rows.max(0)
                tR = ((bmin + bmax) * np.float32(0.5)).astype(np.float32)
                rl = (rows - tR).astype(np.float32)
                cand = cands[R][s0 : s0 + w]
                cl = (cs[cand] - tR).astype(np.float32)
                ri = (
                    (rl[:, 0] * rl[:, 0] + rl[:, 1] * rl[:, 1]) + rl[:, 2] * rl[:, 2]
                ).astype(np.float32)
                rj = (
                    (cl[:, 0] * cl[:, 0] + cl[:, 1] * cl[:, 1]) + cl[:, 2] * cl[:, 2]
                ).astype(np.float32)
                kb = band * KB
                rcol = slice(off, off + w)
                NC = len(KEEP)
                for ci in range(3):
                    rs = _split2(rl[:, ci].copy())
                    csp = _split2(cl[:, ci].copy())
                    for i, (a, bb) in enumerate(KEEP):
                        wa = (np.float32(-2.0) * rs[a]).astype(bf).astype(np.float32)
                        row = kb + ci * NC + i
                        wts_m[row, :] = wa
                        rhs_m[row, rcol] = csp[bb]
                for a, sp in enumerate(_split3(ri.copy())):
                    wts_m[kb + 3 * NC + a, :] = sp
                    rhs_m[kb + 3 * NC + a, rcol] = 1.0
                for bb, sp in enumerate(_split3(rj.copy())):
                    wts_m[kb + 3 * NC + 3 + bb, :] = 1.0
                    rhs_m[kb + 3 * NC + 3 + bb, rcol] = sp
                # scatter indices: orig (hi, lo) pair -> tril slot; self -> scratch
                ro = pi[R * P : (R + 1) * P]
                co = pi[cand]
                hi = np.maximum(ro[:, None], co[None, :])
                lo = np.minimum(ro[:, None], co[None, :])
                flat = hi * N + lo
                flat[ro[:, None] == co[None, :]] = N * N
                idx_m[p, :, off : off + w] = flat
                off += w
        in_maps.append({"inp": np.ascontiguousarray(inp_m).astype(bf)})
        idx_maps.append(idx_m)
    with _lock:
        _cache["cuthi"] = float(cuthi)
        _cache["widths"] = tuple(widths)
    return in_maps, idx_maps, NP


def _prepare_inputs(species, coordinates, cutoff):
    in_maps, idx_maps, NP = _prepare(species, coordinates, cutoff)
    return in_maps


def _run(in_maps, trace=False):
    from concourse import bass_utils

    NP = in_maps[0]["inp"].shape[1] // (P + W)
    with _lock:
        cuthi = _cache["cuthi"]
        widths = _cache["widths"]
    nc = _get_program(NP, widths, cuthi)
    return bass_utils.run_bass_kernel_spmd(
        nc, in_maps, core_ids=list(range(NCORES)), trace=trace
    )


def _assemble(results, idx_maps):
    full = np.zeros(N * N + 1, np.float32)
    for c in range(NCORES):
        vals = results[c]["out"].astype(np.float32)
        full[idx_maps[c].ravel()] = vals.ravel()
    return full[: N * N].reshape(N, N)


def kernel(species, coordinates, cutoff):
    in_maps, idx_maps, NP = _prepare(species, coordinates, cutoff)
    res = _run(in_maps)
    return _assemble(res.results, idx_maps)
